# revision 20
# baseline (speedup 1.0000x reference)
"""Trainium2 Bass kernel for nn_AttentionBlock_15693810500077.

GroupNorm(32 groups) -> 1x1 qkv conv -> 4-head attention (T=4096) ->
1x1 proj -> residual, for x [2, 256, 16, 16, 16] fp32.

Sharding: 8 cores = (batch b in {0,1}) x (t-slice i in {0..3}, TS=1024).
Each core computes the full attention rows for its t-slice of its batch,
for all 4 heads, plus the projection and residual -> y^T slab [1024, 256].
The host rotates each core's x copy (np.roll over T) so the core's t-slice
always sits at columns 0:1024 -> one static SPMD program for all cores
(softmax over keys is permutation invariant).

v2: ACT-exp is the roofline (~147us); PE work is packed under it via
PE-array tiling: score matmuls for a head pair run concurrently as row
tiles (contraction 64 at array rows 0-63 / 64-127), PV matmuls run
concurrently as col tiles (outputs at PSUM partitions 0-63 / 64-127).
Softmax denominators accumulate on DVE in bf16 and are normalized after
the projection with per-partition scalars (no 8-cycle/elem reciprocal of
a broadcast tile on the critical path).

Self-contained: hardcodes all shapes; only needs numpy + the concourse
(Bass) runtime available in the environment.
"""
import os

import numpy as np

os.environ.setdefault("JAX_COMPILATION_CACHE_DIR", "/tmp/jaxcache")

import concourse.bass as bass
import concourse.tile as tile
from concourse import mybir
from concourse.bass_utils import run_bass_kernel_spmd

F32 = mybir.dt.float32
BF16 = mybir.dt.bfloat16
AF = mybir.ActivationFunctionType
ALU = mybir.AluOpType

H = 4
C = 256
T = 4096
TS = 1024
EPS = 1e-5
SCALE2 = 0.125           # (1/sqrt(sqrt(64)))^2
NCH = T // 128           # 32 key chunks of 128


def build_nc():
    nc = bass.Bass()

    x_d = nc.dram_tensor("x", [C, T], F32, kind="ExternalInput")
    xTb_d = nc.dram_tensor("xTb", [128, 8 * C], F32, kind="ExternalInput")
    wqT_d = nc.dram_tensor("wqT", [C, C], BF16, kind="ExternalInput")
    wkT_d = nc.dram_tensor("wkT", [C, C], BF16, kind="ExternalInput")
    wvT_d = nc.dram_tensor("wvT", [C, C], BF16, kind="ExternalInput")
    pTp_d = nc.dram_tensor("pTp", [2, 128, C], BF16, kind="ExternalInput")
    normw_d = nc.dram_tensor("normw", [2, 128, 1], F32, kind="ExternalInput")
    normb_d = nc.dram_tensor("normb", [2, 128, 1], F32, kind="ExternalInput")
    sel_d = nc.dram_tensor("sel", [128, 16], F32, kind="ExternalInput")
    exp_d = nc.dram_tensor("expand", [16, 128], F32, kind="ExternalInput")
    onesb_d = nc.dram_tensor("onesb", [128, 1], BF16, kind="ExternalInput")
    onesf_d = nc.dram_tensor("onesf", [128, 1], F32, kind="ExternalInput")
    sel2_d = nc.dram_tensor("sel2", [33, 2], F32, kind="ExternalInput")
    yT_d = nc.dram_tensor("yT", [TS, C], F32, kind="ExternalOutput")

    import contextlib

    with tile.TileContext(nc) as tc:
        with (
            tc.tile_pool(name="consts", bufs=1) as consts,
            tc.tile_pool(name="gnp", bufs=2) as gnp,
            tc.tile_pool(name="kqv", bufs=1) as kqv,
            # PSUM: psQ = 2 rotating 4KB slots (4 banks), psV = pv pair (4)
            tc.tile_pool(name="psQ", bufs=2, space="PSUM") as psQ,
            tc.tile_pool(name="psV", bufs=1, space="PSUM") as psV,
            contextlib.ExitStack() as late,
        ):
            # ---- constant / weight loads ----
            wq = [consts.tile([128, C], BF16, name=f"wq{i}") for i in range(2)]
            wk = [consts.tile([128, C], BF16, name=f"wk{i}") for i in range(2)]
            wv = [consts.tile([128, C], BF16, name=f"wv{i}") for i in range(2)]
            for i in range(2):
                nc.sync.dma_start(out=wq[i], in_=wqT_d[i * 128:(i + 1) * 128, :])
                nc.sync.dma_start(out=wk[i], in_=wkT_d[i * 128:(i + 1) * 128, :])
                nc.sync.dma_start(out=wv[i], in_=wvT_d[i * 128:(i + 1) * 128, :])
            pTh = [consts.tile([64, C], BF16, name=f"pTh{h}") for h in range(4)]
            for p in range(2):
                nc.sync.dma_start(out=pTh[2 * p], in_=pTp_d[p, 0:64, :])
                nc.sync.dma_start(out=pTh[2 * p + 1], in_=pTp_d[p, 64:128, :])
            normw = [consts.tile([128, 1], F32, name=f"nw{i}") for i in range(2)]
            normb = [consts.tile([128, 1], F32, name=f"nb{i}") for i in range(2)]
            for i in range(2):
                nc.sync.dma_start(out=normw[i], in_=normw_d[i])
                nc.sync.dma_start(out=normb[i], in_=normb_d[i])
            sel = consts.tile([128, 16], F32, name="sel")
            nc.sync.dma_start(out=sel, in_=sel_d[:])
            expand = consts.tile([16, 128], F32, name="expand")
            nc.sync.dma_start(out=expand, in_=exp_d[:])
            onesb = consts.tile([128, 1], BF16, name="onesb")
            nc.sync.dma_start(out=onesb, in_=onesb_d[:])
            onesf = consts.tile([128, 1], F32, name="onesf")
            nc.sync.dma_start(out=onesf, in_=onesf_d[:])
            sel2 = consts.tile([33, 2], F32, name="sel2")
            nc.sync.dma_start(out=sel2, in_=sel2_d[:])

            # ---- x loads after the small consts (per-chunk tiles so GN
            # stats start as soon as the first chunk lands) ----
            xp = late.enter_context(tc.tile_pool(name="xp", bufs=1))
            xt = [[xp.tile([128, 1024], F32, name=f"x{i}_{jc}")
                   for jc in range(4)] for i in range(2)]
            for i in range(2):
                for jc in range(4):
                    nc.sync.dma_start(
                        out=xt[i][jc],
                        in_=x_d[i * 128:(i + 1) * 128,
                                jc * 1024:(jc + 1) * 1024],
                    )
            # residual slab is only needed by the epilogue: queue it last
            xTb_sb = consts.tile([128, 8, C], F32, name="xTb_sb")
            nc.sync.dma_start(
                out=xTb_sb, in_=xTb_d.rearrange("p (a o) -> p a o", o=C)
            )

            # ---- GroupNorm -> xn (bf16) ----
            xn = [kqv.tile([128, T], BF16, name=f"xn{i}") for i in range(2)]
            if True:
                for i in range(2):
                    stats = gnp.tile([128, 8, 6], F32, name="stats", tag="stats")
                    for j in range(8):
                        xv = xt[i][j // 2].rearrange("p (a f) -> p a f", f=512)
                        nc.vector.bn_stats(out=stats[:, j, :],
                                           in_=xv[:, j % 2, :])
                    mv = gnp.tile([128, 2], F32, name="mv", tag="mv")
                    nc.vector.bn_aggr(out=mv, in_=stats)
                    # exsq = var + mean^2
                    msq = gnp.tile([128, 1], F32, name="msq", tag="msq")
                    nc.vector.tensor_mul(msq, mv[:, 0:1], mv[:, 0:1])
                    exsq = gnp.tile([128, 1], F32, name="exsq", tag="exsq")
                    nc.vector.tensor_add(exsq, msq, mv[:, 1:2])
                    # group stats via selector matmuls (tiny)
                    gm_ps = psQ.tile([16, 1], F32, name="gm_ps", tag="qk")
                    nc.tensor.matmul(gm_ps, sel, mv[:, 0:1], start=True, stop=True)
                    gx_ps = psQ.tile([16, 1], F32, name="gx_ps", tag="qk")
                    nc.tensor.matmul(gx_ps, sel, exsq, start=True, stop=True)
                    gm_sb = gnp.tile([16, 1], F32, name="gm_sb", tag="gm_sb")
                    nc.vector.tensor_copy(gm_sb, gm_ps)
                    gmsq = gnp.tile([16, 1], F32, name="gmsq", tag="gmsq")
                    nc.vector.tensor_mul(gmsq, gm_sb, gm_sb)
                    gvar = gnp.tile([16, 1], F32, name="gvar", tag="gvar")
                    nc.vector.scalar_tensor_tensor(
                        gvar, gx_ps, EPS, gmsq, op0=ALU.add, op1=ALU.subtract
                    )
                    # rstd = exp(-0.5 * ln(var + eps))
                    lnv = gnp.tile([16, 1], F32, name="lnv", tag="lnv")
                    nc.scalar.activation(lnv, gvar, AF.Ln)
                    rstd = gnp.tile([16, 1], F32, name="rstd", tag="rstd")
                    nc.scalar.activation(rstd, lnv, AF.Exp, scale=-0.5)
                    # expand to channels
                    me_ps = psQ.tile([128, 1], F32, name="me_ps", tag="qk")
                    nc.tensor.matmul(me_ps, expand, gm_sb, start=True, stop=True)
                    re_ps = psQ.tile([128, 1], F32, name="re_ps", tag="qk")
                    nc.tensor.matmul(re_ps, expand, rstd, start=True, stop=True)
                    a_sb = gnp.tile([128, 1], F32, name="a_sb", tag="a_sb")
                    nc.vector.tensor_mul(a_sb, re_ps, normw[i])
                    t2 = gnp.tile([128, 1], F32, name="t2", tag="t2")
                    nc.vector.tensor_mul(t2, me_ps, a_sb)
                    b_sb = gnp.tile([128, 1], F32, name="b_sb", tag="b_sb")
                    nc.vector.tensor_sub(b_sb, normb[i], t2)
                    # xn in 4 column chunks for finer downstream pipelining
                    for qc in range(4):
                        sl = slice(qc * 1024, (qc + 1) * 1024)
                        nc.vector.tensor_scalar(
                            out=xn[i][:, sl], in0=xt[i][qc],
                            scalar1=a_sb, scalar2=b_sb,
                            op0=ALU.mult, op1=ALU.add,
                        )

            # ---- late pools (opened after the x pool is released) ----
            ppool = late.enter_context(tc.tile_pool(name="ppool", bufs=6))
            stkp = late.enter_context(tc.tile_pool(name="stkp", bufs=2))
            rsp = late.enter_context(tc.tile_pool(name="rsp", bufs=2))
            outp = late.enter_context(tc.tile_pool(name="outp", bufs=1))

            # ---- qkv ----
            # q: pair-stacked [128(2x64ch), TS] per pair
            q_sb = [kqv.tile([128, TS], BF16, name=f"q{o}") for o in range(2)]
            k_sb = [kqv.tile([128, T], BF16, name=f"k{o}") for o in range(2)]
            vTa = kqv.tile([128, H, NCH, 65], BF16, name="vTa")
            nc.vector.memset(vTa[:, :, :, 64:65], 1.0)
            for o in range(2):
                q_ps = psQ.tile([128, TS], F32, name="q_ps", tag="qk")
                for cc in range(2):
                    for n0 in range(0, TS, 512):
                        nc.tensor.matmul(
                            q_ps[:, n0:n0 + 512],
                            wq[cc][:, o * 128:(o + 1) * 128],
                            xn[cc][:, n0:n0 + 512],
                            start=(cc == 0), stop=(cc == 1),
                        )
                nc.vector.tensor_copy(q_sb[o], q_ps)
            for o in range(2):
                for nk in range(8):
                    k_ps = psQ.tile([128, 512], F32, name="k_ps", tag="qk")
                    for cc in range(2):
                        nc.tensor.matmul(
                            k_ps,
                            wk[cc][:, o * 128:(o + 1) * 128],
                            xn[cc][:, nk * 512:(nk + 1) * 512],
                            start=(cc == 0), stop=(cc == 1),
                        )
                    nc.vector.tensor_copy(k_sb[o][:, nk * 512:(nk + 1) * 512], k_ps)
            for tci in range(NCH):
                vt_ps = psQ.tile([128, C], F32, name="vt_ps", tag="qk")
                for cc in range(2):
                    nc.tensor.matmul(
                        vt_ps,
                        xn[cc][:, tci * 128:(tci + 1) * 128],
                        wv[cc],
                        start=(cc == 0), stop=(cc == 1),
                    )
                nc.vector.tensor_copy(
                    vTa[:, :, tci, 0:64],
                    vt_ps.rearrange("p (h c) -> p h c", h=H),
                )

            # ---- attention: 2 head pairs; row/col-tiled concurrent MMs ----
            out_sb = outp.tile([128, 8, C], F32, name="out_sb")

            def epilogue(pair, pvp):
                """Normalize + proj + residual for one head pair."""
                stkuA = stkp.tile([64, TS], BF16, name="stkuA", tag="stkuA")
                stkuB = stkp.tile([64, TS], BF16, name="stkuB", tag="stkuB")
                nc.vector.tensor_copy(stkuA, pvp[0:64, 0:TS])
                nc.vector.tensor_copy(stkuB, pvp[0:64, TS:2 * TS])
                # rowsums sit in pvp row 64 of each half (ones-row trick);
                # park them on partitions 0 and 32 of one tile and transpose
                # both heads per t-chunk with a single K=33 matmul against a
                # selector that zeroes the garbage rows in between.
                rs2 = rsp.tile([33, TS], F32, name="rs2", tag="rs")
                nc.vector.tensor_copy(rs2[0:1, :], pvp[64:65, 0:TS])
                nc.vector.tensor_copy(rs2[32:33, :], pvp[64:65, TS:2 * TS])
                rsT_ps = psQ.tile([128, 8, 2], F32, name="rsT_ps", tag="qk")
                for tc_i in range(8):
                    tsl = slice(tc_i * 128, (tc_i + 1) * 128)
                    nc.tensor.matmul(
                        rsT_ps[:, tc_i, :], rs2[:, tsl], sel2,
                        start=True, stop=True,
                    )
                recipT = rsp.tile([128, 8, 2], F32, name="recipT", tag="recipT")
                nc.vector.reciprocal(recipT, rsT_ps)
                # proj (row-tiled head pair) + normalize + residual
                for tc_i in range(8):
                    tsl = slice(tc_i * 128, (tc_i + 1) * 128)
                    pr = psQ.tile([128, 2, C], F32, name="pr", tag="qk")
                    nc.tensor.matmul(
                        pr[:, 0, :], stkuA[:, tsl], pTh[2 * pair],
                        start=True, stop=True,
                    )
                    nc.tensor.matmul(
                        pr[:, 1, :], stkuB[:, tsl], pTh[2 * pair + 1],
                        start=True, stop=True,
                    )
                    base = xTb_sb[:, tc_i, :] if pair == 0 else out_sb[:, tc_i, :]
                    nc.vector.scalar_tensor_tensor(
                        out_sb[:, tc_i, :], pr[:, 0, :],
                        recipT[:, tc_i, 0:1], base,
                        op0=ALU.mult, op1=ALU.add,
                    )
                    nc.vector.scalar_tensor_tensor(
                        out_sb[:, tc_i, :], pr[:, 1, :],
                        recipT[:, tc_i, 1:2], out_sb[:, tc_i, :],
                        op0=ALU.mult, op1=ALU.add,
                    )
                    if pair == 1:
                        nc.sync.dma_start(
                            out=yT_d[tc_i * 128:(tc_i + 1) * 128, :],
                            in_=out_sb[:, tc_i, :],
                        )

            pend = None  # deferred (pair, pvp) epilogue args
            for pair in range(2):
                # A accumulates in [0:64, 0:TS] (banks 0-1), B in
                # [64:128, TS:2*TS] (banks 2-3): col-tiled concurrency
                # without sharing a PSUM zero region between groups.
                pvp = psV.tile([128, 2 * TS], F32, name="pvp", tag="pv")
                kt, qt = k_sb[pair], q_sb[pair]
                for sc in range(NCH):
                    if pend is not None and sc == 5:
                        epilogue(*pend)
                        pend = None
                    ksl = slice(sc * 128, (sc + 1) * 128)
                    qkA = psQ.tile([128, TS], F32, name="qkA", tag="qk")
                    qkB = psQ.tile([128, TS], F32, name="qkB", tag="qk")
                    for n0 in range(0, TS, 512):
                        nc.tensor.matmul(
                            qkA[:, n0:n0 + 512], kt[0:64, ksl],
                            qt[0:64, n0:n0 + 512], start=True, stop=True,
                        )
                    for n0 in range(0, TS, 512):
                        nc.tensor.matmul(
                            qkB[:, n0:n0 + 512], kt[64:128, ksl],
                            qt[64:128, n0:n0 + 512], start=True, stop=True,
                        )
                    pA = ppool.tile([128, TS], BF16, name="p_t", tag="p")
                    nc.scalar.activation(pA, qkA, AF.Exp, scale=SCALE2)
                    pB = ppool.tile([128, TS], BF16, name="p_t", tag="p")
                    nc.scalar.activation(pB, qkB, AF.Exp, scale=SCALE2)
                    # PV with ones row (col 64): rowsum accumulates in row 64
                    vA = vTa[:, 2 * pair, sc, :]
                    vB = vTa[:, 2 * pair + 1, sc, :]
                    for n0 in range(0, TS, 512):
                        nc.tensor.matmul(
                            pvp[0:65, n0:n0 + 512], vA, pA[:, n0:n0 + 512],
                            start=(sc == 0), stop=(sc == NCH - 1),
                        )
                    for n0 in range(0, TS, 512):
                        nc.tensor.matmul(
                            pvp[0:65, TS + n0:TS + n0 + 512], vB,
                            pB[:, n0:n0 + 512],
                            start=(sc == 0), stop=(sc == NCH - 1),
                        )
                pend = (pair, pvp)
            epilogue(*pend)

    # Legalize for this walrus: at most 1 sync wait per instruction.
    import bass_rust as _bass_rust
    _bass_rust.move_matmul_waits_to_ldweights(nc.m)
    _bass_rust.generate_event_semaphores(nc)
    return nc


def host_prep(inputs):
    """Per-core input dicts (pure slicing / transpose / permutation)."""
    x = np.ascontiguousarray(np.asarray(inputs["x"], np.float32).reshape(2, C, T))
    qkv_w = np.asarray(inputs["qkv_w"], np.float32)
    proj_w = np.asarray(inputs["proj_w"], np.float32)
    norm_w = np.ascontiguousarray(np.asarray(inputs["norm_w"], np.float32))
    norm_b = np.ascontiguousarray(np.asarray(inputs["norm_b"], np.float32))
    proj_b = np.ascontiguousarray(np.asarray(inputs["proj_b"], np.float32))

    q_idx = np.concatenate([np.arange(h * 192, h * 192 + 64) for h in range(H)])
    wqT = np.ascontiguousarray(qkv_w[q_idx].T)
    wkT = np.ascontiguousarray(qkv_w[q_idx + 64].T)
    wvT = np.ascontiguousarray(qkv_w[q_idx + 128].T)
    pT = proj_w.T.reshape(4, 64, C)
    pTp = np.ascontiguousarray(pT.reshape(2, 128, C))

    sel2 = np.zeros((33, 2), np.float32)
    sel2[0, 0] = 1.0
    sel2[32, 1] = 1.0
    sel = np.zeros((128, 16), np.float32)
    sel[np.arange(128), np.arange(128) // 8] = 1.0 / 8.0
    expand = np.zeros((16, 128), np.float32)
    expand[np.arange(128) // 8, np.arange(128)] = 1.0

    bf = __import__("ml_dtypes").bfloat16
    shared = {
        "wqT": wqT.astype(bf), "wkT": wkT.astype(bf), "wvT": wvT.astype(bf),
        "pTp": pTp.astype(bf),
        "normw": np.ascontiguousarray(norm_w.reshape(2, 128, 1)),
        "normb": np.ascontiguousarray(norm_b.reshape(2, 128, 1)),
        "sel": sel, "expand": expand,
        "onesb": np.ones((128, 1), np.float32).astype(bf),
        "onesf": np.ones((128, 1), np.float32),
        "sel2": sel2,
    }
    in_maps = []
    for core in range(8):
        b, i = core // 4, core % 4
        t0 = i * TS
        m = dict(shared)
        m["x"] = np.ascontiguousarray(np.roll(x[b], -t0, axis=1))
        xTb = x[b, :, t0:t0 + TS].T + proj_b[None, :]
        m["xTb"] = np.ascontiguousarray(
            xTb.reshape(8, 128, C).transpose(1, 0, 2).reshape(128, 8 * C)
        )
        in_maps.append(m)
    return in_maps


def gather(core_outs):
    y = np.empty((2, C, T), np.float32)
    for core in range(8):
        b, i = core // 4, core % 4
        y[b, :, i * TS:(i + 1) * TS] = core_outs[core].T
    return y.reshape(2, C, 16, 16, 16)


_NC = None


def _get_nc():
    global _NC
    if _NC is None:
        _NC = build_nc()
    return _NC


def run(inputs, trace=False, trace_cores=None):
    nc = _get_nc()
    in_maps = host_prep(inputs)
    res = run_bass_kernel_spmd(
        nc, in_maps, list(range(8)), trace=trace, trace_cores=trace_cores
    )
    out = gather([res.results[c]["yT"] for c in range(8)])
    return out, res


def kernel(**inputs) -> np.ndarray:
    out, _ = run(inputs)
    return out


# revision 21
# speedup vs baseline: 1.0237x; 1.0237x over previous
"""Trainium2 Bass kernel for nn_AttentionBlock_15693810500077.

GroupNorm(32 groups) -> 1x1 qkv conv -> 4-head attention (T=4096) ->
1x1 proj -> residual, for x [2, 256, 16, 16, 16] fp32.

Sharding: 8 cores = (batch b in {0,1}) x (t-slice i in {0..3}, TS=1024).
Each core computes the full attention rows for its t-slice of its batch,
for all 4 heads, plus the projection and residual -> y^T slab [1024, 256].
The host rotates each core's x copy (np.roll over T) so the core's t-slice
always sits at columns 0:1024 -> one static SPMD program for all cores
(softmax over keys is permutation invariant).

v2: ACT-exp is the roofline (~147us); PE work is packed under it via
PE-array tiling: score matmuls for a head pair run concurrently as row
tiles (contraction 64 at array rows 0-63 / 64-127), PV matmuls run
concurrently as col tiles (outputs at PSUM partitions 0-63 / 64-127).
Softmax denominators accumulate on DVE in bf16 and are normalized after
the projection with per-partition scalars (no 8-cycle/elem reciprocal of
a broadcast tile on the critical path).

Self-contained: hardcodes all shapes; only needs numpy + the concourse
(Bass) runtime available in the environment.
"""
import os

import numpy as np

os.environ.setdefault("JAX_COMPILATION_CACHE_DIR", "/tmp/jaxcache")

import concourse.bass as bass
import concourse.tile as tile
from concourse import mybir
from concourse.bass_utils import run_bass_kernel_spmd

F32 = mybir.dt.float32
BF16 = mybir.dt.bfloat16
AF = mybir.ActivationFunctionType
ALU = mybir.AluOpType

H = 4
C = 256
T = 4096
TS = 1024
EPS = 1e-5
SCALE2 = 0.125           # (1/sqrt(sqrt(64)))^2
NCH = T // 128           # 32 key chunks of 128


def build_nc():
    nc = bass.Bass()

    x_d = nc.dram_tensor("x", [C, T], F32, kind="ExternalInput")
    xTb_d = nc.dram_tensor("xTb", [128, 8 * C], F32, kind="ExternalInput")
    wqT_d = nc.dram_tensor("wqT", [C, C], BF16, kind="ExternalInput")
    wkT_d = nc.dram_tensor("wkT", [C, C], BF16, kind="ExternalInput")
    wvT_d = nc.dram_tensor("wvT", [C, C], BF16, kind="ExternalInput")
    pTp_d = nc.dram_tensor("pTp", [2, 128, C], BF16, kind="ExternalInput")
    normw_d = nc.dram_tensor("normw", [2, 128, 1], F32, kind="ExternalInput")
    normb_d = nc.dram_tensor("normb", [2, 128, 1], F32, kind="ExternalInput")
    sel_d = nc.dram_tensor("sel", [128, 16], F32, kind="ExternalInput")
    exp_d = nc.dram_tensor("expand", [16, 128], F32, kind="ExternalInput")
    onesb_d = nc.dram_tensor("onesb", [128, 1], BF16, kind="ExternalInput")
    onesf_d = nc.dram_tensor("onesf", [128, 1], F32, kind="ExternalInput")
    yT_d = nc.dram_tensor("yT", [TS, C], F32, kind="ExternalOutput")

    import contextlib

    with tile.TileContext(nc) as tc:
        with (
            tc.tile_pool(name="consts", bufs=1) as consts,
            tc.tile_pool(name="gnp", bufs=2) as gnp,
            tc.tile_pool(name="kqv", bufs=1) as kqv,
            # PSUM: psQ = 2 rotating 4KB slots (4 banks), psV = pv pair (4)
            tc.tile_pool(name="psQ", bufs=2, space="PSUM") as psQ,
            tc.tile_pool(name="psV", bufs=1, space="PSUM") as psV,
            contextlib.ExitStack() as late,
        ):
            # ---- x loads first (per-chunk tiles so GN stats can start as
            # soon as the first chunk lands, not after the whole tensor) ----
            xp = late.enter_context(tc.tile_pool(name="xp", bufs=1))
            xt = [[xp.tile([128, 1024], F32, name=f"x{i}_{jc}")
                   for jc in range(4)] for i in range(2)]
            for i in range(2):
                for jc in range(4):
                    nc.sync.dma_start(
                        out=xt[i][jc],
                        in_=x_d[i * 128:(i + 1) * 128,
                                jc * 1024:(jc + 1) * 1024],
                    )

            # ---- constant / weight loads ----
            wq = [consts.tile([128, C], BF16, name=f"wq{i}") for i in range(2)]
            wk = [consts.tile([128, C], BF16, name=f"wk{i}") for i in range(2)]
            wv = [consts.tile([128, C], BF16, name=f"wv{i}") for i in range(2)]
            for i in range(2):
                nc.sync.dma_start(out=wq[i], in_=wqT_d[i * 128:(i + 1) * 128, :])
                nc.sync.dma_start(out=wk[i], in_=wkT_d[i * 128:(i + 1) * 128, :])
                nc.sync.dma_start(out=wv[i], in_=wvT_d[i * 128:(i + 1) * 128, :])
            pTh = [consts.tile([64, C], BF16, name=f"pTh{h}") for h in range(4)]
            for p in range(2):
                nc.sync.dma_start(out=pTh[2 * p], in_=pTp_d[p, 0:64, :])
                nc.sync.dma_start(out=pTh[2 * p + 1], in_=pTp_d[p, 64:128, :])
            normw = [consts.tile([128, 1], F32, name=f"nw{i}") for i in range(2)]
            normb = [consts.tile([128, 1], F32, name=f"nb{i}") for i in range(2)]
            for i in range(2):
                nc.sync.dma_start(out=normw[i], in_=normw_d[i])
                nc.sync.dma_start(out=normb[i], in_=normb_d[i])
            sel = consts.tile([128, 16], F32, name="sel")
            nc.sync.dma_start(out=sel, in_=sel_d[:])
            expand = consts.tile([16, 128], F32, name="expand")
            nc.sync.dma_start(out=expand, in_=exp_d[:])
            onesb = consts.tile([128, 1], BF16, name="onesb")
            nc.sync.dma_start(out=onesb, in_=onesb_d[:])
            onesf = consts.tile([128, 1], F32, name="onesf")
            nc.sync.dma_start(out=onesf, in_=onesf_d[:])
            xTb_sb = consts.tile([128, 8, C], F32, name="xTb_sb")
            nc.sync.dma_start(
                out=xTb_sb, in_=xTb_d.rearrange("p (a o) -> p a o", o=C)
            )

            # ---- GroupNorm -> xn (bf16) ----
            xn = [kqv.tile([128, T], BF16, name=f"xn{i}") for i in range(2)]
            if True:
                for i in range(2):
                    stats = gnp.tile([128, 8, 6], F32, name="stats", tag="stats")
                    for j in range(8):
                        xv = xt[i][j // 2].rearrange("p (a f) -> p a f", f=512)
                        nc.vector.bn_stats(out=stats[:, j, :],
                                           in_=xv[:, j % 2, :])
                    mv = gnp.tile([128, 2], F32, name="mv", tag="mv")
                    nc.vector.bn_aggr(out=mv, in_=stats)
                    # exsq = var + mean^2
                    msq = gnp.tile([128, 1], F32, name="msq", tag="msq")
                    nc.vector.tensor_mul(msq, mv[:, 0:1], mv[:, 0:1])
                    exsq = gnp.tile([128, 1], F32, name="exsq", tag="exsq")
                    nc.vector.tensor_add(exsq, msq, mv[:, 1:2])
                    # group stats via selector matmuls (tiny)
                    gm_ps = psQ.tile([16, 1], F32, name="gm_ps", tag="qk")
                    nc.tensor.matmul(gm_ps, sel, mv[:, 0:1], start=True, stop=True)
                    gx_ps = psQ.tile([16, 1], F32, name="gx_ps", tag="qk")
                    nc.tensor.matmul(gx_ps, sel, exsq, start=True, stop=True)
                    gm_sb = gnp.tile([16, 1], F32, name="gm_sb", tag="gm_sb")
                    nc.vector.tensor_copy(gm_sb, gm_ps)
                    gmsq = gnp.tile([16, 1], F32, name="gmsq", tag="gmsq")
                    nc.vector.tensor_mul(gmsq, gm_sb, gm_sb)
                    gvar = gnp.tile([16, 1], F32, name="gvar", tag="gvar")
                    nc.vector.scalar_tensor_tensor(
                        gvar, gx_ps, EPS, gmsq, op0=ALU.add, op1=ALU.subtract
                    )
                    # rstd = exp(-0.5 * ln(var + eps))
                    lnv = gnp.tile([16, 1], F32, name="lnv", tag="lnv")
                    nc.scalar.activation(lnv, gvar, AF.Ln)
                    rstd = gnp.tile([16, 1], F32, name="rstd", tag="rstd")
                    nc.scalar.activation(rstd, lnv, AF.Exp, scale=-0.5)
                    # expand to channels
                    me_ps = psQ.tile([128, 1], F32, name="me_ps", tag="qk")
                    nc.tensor.matmul(me_ps, expand, gm_sb, start=True, stop=True)
                    re_ps = psQ.tile([128, 1], F32, name="re_ps", tag="qk")
                    nc.tensor.matmul(re_ps, expand, rstd, start=True, stop=True)
                    a_sb = gnp.tile([128, 1], F32, name="a_sb", tag="a_sb")
                    nc.vector.tensor_mul(a_sb, re_ps, normw[i])
                    t2 = gnp.tile([128, 1], F32, name="t2", tag="t2")
                    nc.vector.tensor_mul(t2, me_ps, a_sb)
                    b_sb = gnp.tile([128, 1], F32, name="b_sb", tag="b_sb")
                    nc.vector.tensor_sub(b_sb, normb[i], t2)
                    # xn in 4 column chunks for finer downstream pipelining
                    for qc in range(4):
                        sl = slice(qc * 1024, (qc + 1) * 1024)
                        nc.vector.tensor_scalar(
                            out=xn[i][:, sl], in0=xt[i][qc],
                            scalar1=a_sb, scalar2=b_sb,
                            op0=ALU.mult, op1=ALU.add,
                        )

            # ---- late pools (opened after the x pool is released) ----
            ppool = late.enter_context(tc.tile_pool(name="ppool", bufs=6))
            stkp = late.enter_context(tc.tile_pool(name="stkp", bufs=2))
            rsp = late.enter_context(tc.tile_pool(name="rsp", bufs=2))
            outp = late.enter_context(tc.tile_pool(name="outp", bufs=1))

            # ---- qkv ----
            # q: pair-stacked [128(2x64ch), TS] per pair
            q_sb = [kqv.tile([128, TS], BF16, name=f"q{o}") for o in range(2)]
            k_sb = [kqv.tile([128, T], BF16, name=f"k{o}") for o in range(2)]
            vTa = kqv.tile([128, H, NCH, 65], BF16, name="vTa")
            nc.vector.memset(vTa[:, :, :, 64:65], 1.0)
            for o in range(2):
                q_ps = psQ.tile([128, TS], F32, name="q_ps", tag="qk")
                for cc in range(2):
                    for n0 in range(0, TS, 512):
                        nc.tensor.matmul(
                            q_ps[:, n0:n0 + 512],
                            wq[cc][:, o * 128:(o + 1) * 128],
                            xn[cc][:, n0:n0 + 512],
                            start=(cc == 0), stop=(cc == 1),
                        )
                nc.vector.tensor_copy(q_sb[o], q_ps)
            for o in range(2):
                for nk in range(8):
                    k_ps = psQ.tile([128, 512], F32, name="k_ps", tag="qk")
                    for cc in range(2):
                        nc.tensor.matmul(
                            k_ps,
                            wk[cc][:, o * 128:(o + 1) * 128],
                            xn[cc][:, nk * 512:(nk + 1) * 512],
                            start=(cc == 0), stop=(cc == 1),
                        )
                    nc.vector.tensor_copy(k_sb[o][:, nk * 512:(nk + 1) * 512], k_ps)
            for tci in range(NCH):
                vt_ps = psQ.tile([128, C], F32, name="vt_ps", tag="qk")
                for cc in range(2):
                    nc.tensor.matmul(
                        vt_ps,
                        xn[cc][:, tci * 128:(tci + 1) * 128],
                        wv[cc],
                        start=(cc == 0), stop=(cc == 1),
                    )
                nc.vector.tensor_copy(
                    vTa[:, :, tci, 0:64],
                    vt_ps.rearrange("p (h c) -> p h c", h=H),
                )

            # ---- attention: 2 head pairs; row/col-tiled concurrent MMs ----
            out_sb = outp.tile([128, 8, C], F32, name="out_sb")

            def epilogue(pair, pvp):
                """Normalize + proj + residual for one head pair."""
                stkuA = stkp.tile([64, TS], BF16, name="stkuA", tag="stkuA")
                stkuB = stkp.tile([64, TS], BF16, name="stkuB", tag="stkuB")
                nc.vector.tensor_copy(stkuA, pvp[0:64, 0:TS])
                nc.vector.tensor_copy(stkuB, pvp[0:64, TS:2 * TS])
                # rowsums sit in pvp row 64 of each half (ones-row trick)
                rs_list = []
                for hh in range(2):
                    rs_sb = rsp.tile([1, TS], F32, name=f"rs_sb{hh}",
                                     tag=f"rs{hh}")
                    nc.vector.tensor_copy(
                        rs_sb, pvp[64:65, hh * TS:(hh + 1) * TS]
                    )
                    rs_list.append(rs_sb)
                # transpose rowsums to [128(t), 8, 2] via tiny N=1 matmuls
                rsT_ps = psQ.tile([128, 8, 2], F32, name="rsT_ps", tag="qk")
                for tc_i in range(8):
                    tsl = slice(tc_i * 128, (tc_i + 1) * 128)
                    for hh in range(2):
                        nc.tensor.matmul(
                            rsT_ps[:, tc_i, hh:hh + 1],
                            rs_list[hh][0:1, tsl],
                            onesf[0:1, :],
                            start=True, stop=True,
                        )
                recipT = rsp.tile([128, 8, 2], F32, name="recipT", tag="recipT")
                nc.vector.reciprocal(recipT, rsT_ps)
                # proj (row-tiled head pair) + normalize + residual
                for tc_i in range(8):
                    tsl = slice(tc_i * 128, (tc_i + 1) * 128)
                    pr = psQ.tile([128, 2, C], F32, name="pr", tag="qk")
                    nc.tensor.matmul(
                        pr[:, 0, :], stkuA[:, tsl], pTh[2 * pair],
                        start=True, stop=True,
                    )
                    nc.tensor.matmul(
                        pr[:, 1, :], stkuB[:, tsl], pTh[2 * pair + 1],
                        start=True, stop=True,
                    )
                    base = xTb_sb[:, tc_i, :] if pair == 0 else out_sb[:, tc_i, :]
                    nc.vector.scalar_tensor_tensor(
                        out_sb[:, tc_i, :], pr[:, 0, :],
                        recipT[:, tc_i, 0:1], base,
                        op0=ALU.mult, op1=ALU.add,
                    )
                    nc.vector.scalar_tensor_tensor(
                        out_sb[:, tc_i, :], pr[:, 1, :],
                        recipT[:, tc_i, 1:2], out_sb[:, tc_i, :],
                        op0=ALU.mult, op1=ALU.add,
                    )
                    if pair == 1:
                        nc.sync.dma_start(
                            out=yT_d[tc_i * 128:(tc_i + 1) * 128, :],
                            in_=out_sb[:, tc_i, :],
                        )

            pend = None  # deferred (pair, pvp) epilogue args
            for pair in range(2):
                # A accumulates in [0:64, 0:TS] (banks 0-1), B in
                # [64:128, TS:2*TS] (banks 2-3): col-tiled concurrency
                # without sharing a PSUM zero region between groups.
                pvp = psV.tile([128, 2 * TS], F32, name="pvp", tag="pv")
                kt, qt = k_sb[pair], q_sb[pair]
                for sc in range(NCH):
                    if pend is not None and sc == 5:
                        epilogue(*pend)
                        pend = None
                    ksl = slice(sc * 128, (sc + 1) * 128)
                    qkA = psQ.tile([128, TS], F32, name="qkA", tag="qk")
                    qkB = psQ.tile([128, TS], F32, name="qkB", tag="qk")
                    for n0 in range(0, TS, 512):
                        nc.tensor.matmul(
                            qkA[:, n0:n0 + 512], kt[0:64, ksl],
                            qt[0:64, n0:n0 + 512], start=True, stop=True,
                        )
                    for n0 in range(0, TS, 512):
                        nc.tensor.matmul(
                            qkB[:, n0:n0 + 512], kt[64:128, ksl],
                            qt[64:128, n0:n0 + 512], start=True, stop=True,
                        )
                    pA = ppool.tile([128, TS], BF16, name="p_t", tag="p")
                    nc.scalar.activation(pA, qkA, AF.Exp, scale=SCALE2)
                    pB = ppool.tile([128, TS], BF16, name="p_t", tag="p")
                    nc.scalar.activation(pB, qkB, AF.Exp, scale=SCALE2)
                    # PV with ones row (col 64): rowsum accumulates in row 64
                    vA = vTa[:, 2 * pair, sc, :]
                    vB = vTa[:, 2 * pair + 1, sc, :]
                    for n0 in range(0, TS, 512):
                        nc.tensor.matmul(
                            pvp[0:65, n0:n0 + 512], vA, pA[:, n0:n0 + 512],
                            start=(sc == 0), stop=(sc == NCH - 1),
                        )
                    for n0 in range(0, TS, 512):
                        nc.tensor.matmul(
                            pvp[0:65, TS + n0:TS + n0 + 512], vB,
                            pB[:, n0:n0 + 512],
                            start=(sc == 0), stop=(sc == NCH - 1),
                        )
                pend = (pair, pvp)
            epilogue(*pend)

    # Legalize for this walrus: at most 1 sync wait per instruction.
    import bass_rust as _bass_rust
    _bass_rust.move_matmul_waits_to_ldweights(nc.m)
    _bass_rust.generate_event_semaphores(nc)
    return nc


def host_prep(inputs):
    """Per-core input dicts (pure slicing / transpose / permutation)."""
    x = np.ascontiguousarray(np.asarray(inputs["x"], np.float32).reshape(2, C, T))
    qkv_w = np.asarray(inputs["qkv_w"], np.float32)
    proj_w = np.asarray(inputs["proj_w"], np.float32)
    norm_w = np.ascontiguousarray(np.asarray(inputs["norm_w"], np.float32))
    norm_b = np.ascontiguousarray(np.asarray(inputs["norm_b"], np.float32))
    proj_b = np.ascontiguousarray(np.asarray(inputs["proj_b"], np.float32))

    q_idx = np.concatenate([np.arange(h * 192, h * 192 + 64) for h in range(H)])
    wqT = np.ascontiguousarray(qkv_w[q_idx].T)
    wkT = np.ascontiguousarray(qkv_w[q_idx + 64].T)
    wvT = np.ascontiguousarray(qkv_w[q_idx + 128].T)
    pT = proj_w.T.reshape(4, 64, C)
    pTp = np.ascontiguousarray(pT.reshape(2, 128, C))

    sel = np.zeros((128, 16), np.float32)
    sel[np.arange(128), np.arange(128) // 8] = 1.0 / 8.0
    expand = np.zeros((16, 128), np.float32)
    expand[np.arange(128) // 8, np.arange(128)] = 1.0

    bf = __import__("ml_dtypes").bfloat16
    shared = {
        "wqT": wqT.astype(bf), "wkT": wkT.astype(bf), "wvT": wvT.astype(bf),
        "pTp": pTp.astype(bf),
        "normw": np.ascontiguousarray(norm_w.reshape(2, 128, 1)),
        "normb": np.ascontiguousarray(norm_b.reshape(2, 128, 1)),
        "sel": sel, "expand": expand,
        "onesb": np.ones((128, 1), np.float32).astype(bf),
        "onesf": np.ones((128, 1), np.float32),
    }
    in_maps = []
    for core in range(8):
        b, i = core // 4, core % 4
        t0 = i * TS
        m = dict(shared)
        m["x"] = np.ascontiguousarray(np.roll(x[b], -t0, axis=1))
        xTb = x[b, :, t0:t0 + TS].T + proj_b[None, :]
        m["xTb"] = np.ascontiguousarray(
            xTb.reshape(8, 128, C).transpose(1, 0, 2).reshape(128, 8 * C)
        )
        in_maps.append(m)
    return in_maps


def gather(core_outs):
    y = np.empty((2, C, T), np.float32)
    for core in range(8):
        b, i = core // 4, core % 4
        y[b, :, i * TS:(i + 1) * TS] = core_outs[core].T
    return y.reshape(2, C, 16, 16, 16)


_NC = None


def _get_nc():
    global _NC
    if _NC is None:
        _NC = build_nc()
    return _NC


def run(inputs, trace=False, trace_cores=None):
    nc = _get_nc()
    in_maps = host_prep(inputs)
    res = run_bass_kernel_spmd(
        nc, in_maps, list(range(8)), trace=trace, trace_cores=trace_cores
    )
    out = gather([res.results[c]["yT"] for c in range(8)])
    return out, res


def kernel(**inputs) -> np.ndarray:
    out, _ = run(inputs)
    return out
